# revision 28
# baseline (speedup 1.0000x reference)
"""Multi-head self-attention (B=4, T=2048, C=768, H=12) on 8 trn2 NeuronCores.

Sharding: core c -> batch b=c//2, head-group g=c%2 (6 heads each).
Each core computes its 6 heads' attention and a partial output projection
(contraction over its 384 ctx dims). Host sums the 2 partials per batch
and adds the bias.

v2 dataflow (bf16 compute, f32 PSUM accumulate):
  X -> bf16 via gpsimd cast-DMA -> X^T via DMA-XBAR transpose (no PE)
  qT/kT [128, T] bf16 per pair m (2 heads: partitions 0-63 / 64-127)
  v -> va[m][128, t, g, 65] via DMA transpose; col 0 = ones (softmax
  denominators fall out of the ctx matmul row 0)
  attention per (pair m, q-block qb of 512, key chunk j):
    sps[128, 1024] = [h0 scores | h1 scores]   (two K=64 row-tiled MMs)
    exp on ACT -> pt bf16; ctx MMs accumulate cps_h [65, 512]
  normalize: recip(sums) -> DMA partition-broadcast -> DVE mul -> ctxT bf16
  outproj: psA/psB f32 accumulate over m -> DVE copy -> DMA out (partial)

KERNEL_REPEAT=N builds the body N times (for overhead-cancelling timing).
"""
import sys
import os

sys.path.insert(0, "/opt/trn_rl_repo")

import numpy as np

P = 128
T = 2048
C = 768
HD = 384          # per-core head columns (6 heads x 64)
D = 64
NT = T // P       # 16 T chunks of 128
KC = C // P       # 6 contraction chunks for C
MC = HD // P      # 3 pairs of heads (128 head-dims each)
QB = 512          # q-block size for attention
NQB = T // QB     # 4 q-blocks
VC = 80           # per-head va col block (64 v dims + ones + pad, 32B-aligned)
VW = 2 * VC

_cache = {}


def _build(repeat=1):
    import concourse.bacc as bacc
    import concourse.mybir as mybir
    import concourse.tile as tile
    from concourse.masks import make_identity
    from contextlib import ExitStack

    F32 = mybir.dt.float32
    BF16 = mybir.dt.bfloat16
    AF = mybir.ActivationFunctionType
    ALU = mybir.AluOpType

    nc = bacc.Bacc("TRN2", target_bir_lowering=False, debug=False)
    x = nc.dram_tensor("x", [T, C], F32, kind="ExternalInput").ap()
    wq = nc.dram_tensor("wq", [C, HD], F32, kind="ExternalInput").ap()
    wk = nc.dram_tensor("wk", [C, HD], F32, kind="ExternalInput").ap()
    wv = nc.dram_tensor("wv", [C, HD], F32, kind="ExternalInput").ap()
    wo = nc.dram_tensor("wo", [HD, C], F32, kind="ExternalInput").ap()
    out = nc.dram_tensor("out", [T, C], F32, kind="ExternalOutput").ap()

    def emit(pfx, tc, pools):
        (ident,), big, vap, work, outp, norm, normp = pools

        # ---- X -> bf16 (cast DMA) -> X^T via PE transpose (1 cyc/row bf16)
        # All pre-attention PSUM scratch (X^T, proj, v-transpose) shares one
        # 2-bank pool so the attention pools (sps/cps) never alias banks
        # still held by projection work.
        xt = [big.tile([P, T], BF16, name=f"{pfx}xt{kc}", tag="bigT") for kc in range(KC)]

        def xphase(pool):
            with tc.tile_pool(name=pfx + "xrp", bufs=5) as xrp:
                for tq in range(NT // 4):
                    xrs = []
                    for i in range(4):
                        t_i = 4 * tq + i
                        xr = xrp.tile([P, C], BF16, name=f"{pfx}xr{t_i}", tag="xr")
                        nc.gpsimd.dma_start(xr[:], x[P * t_i:P * (t_i + 1), :])
                        xrs.append(xr)
                    for kc in range(KC):
                        tp = pool.tile([P, 512], BF16, name=f"{pfx}tp_{tq}_{kc}", tag="ps")
                        for i in range(4):
                            nc.tensor.transpose(tp[:, P * i:P * (i + 1)], xrs[i][:, P * kc:P * (kc + 1)], ident[:])
                        nc.vector.tensor_copy(xt[kc][:, 512 * tq:512 * (tq + 1)], tp[:])

        # ---- weights -> bf16 via cast DMA
        w_b = {}
        wo_b = []
        for nm, src in (("q", wq), ("k", wk), ("v", wv)):
            for kc in range(KC):
                t_b = norm.tile([P, HD], BF16, name=f"{pfx}w_{nm}{kc}", tag=f"w_{nm}{kc}")
                nc.gpsimd.dma_start(t_b[:], src[P * kc:P * (kc + 1), :])
                w_b[nm, kc] = t_b
        for m in range(MC):
            t_b = norm.tile([P, C], BF16, name=f"{pfx}wo_{m}", tag=f"wo_{m}")
            nc.gpsimd.dma_start(t_b[:], wo[P * m:P * (m + 1), :])
            wo_b.append(t_b)

        qT = [big.tile([P, T], BF16, name=f"{pfx}qT{m}", tag="bigT") for m in range(MC)]
        kT = [big.tile([P, T], BF16, name=f"{pfx}kT{m}", tag="bigT") for m in range(MC)]
        va = [vap.tile([P, NT * VW], BF16, name=f"{pfx}va{m}", tag=f"va{m}") for m in range(MC)]
        ctxT = [big.tile([P, T], BF16, name=f"{pfx}ctxT{m}", tag="bigT") for m in range(MC)]

        def proj_m(m, pool, vtp, interleave=False):
            """q/k/v projections + va (with ones col at c=D) for pair m.

            interleave=True emits per n-block (k,q,v then va chunk) so the
            first scores/exp of the following attention can start after
            one n-block instead of the whole projection.
            """
            vT = vtp.tile([P, T], BF16, name=f"{pfx}vT{m}", tag="vT")
            vav = va[m][:].rearrange("p (t g c) -> p t g c", t=NT, g=2)
            nc.vector.memset(vav[:, :, :, D:D + 1], 1.0)

            def block(nm, n):
                dest = {"q": qT[m], "k": kT[m], "v": vT}[nm]
                ps = pool.tile([P, 512], F32, name=f"{pfx}ps_{nm}{m}{n}", tag="ps")
                for kc in range(KC):
                    nc.tensor.matmul(
                        ps[:],
                        w_b[nm, kc][:, P * m:P * (m + 1)],
                        xt[kc][:, 512 * n:512 * (n + 1)],
                        start=(kc == 0), stop=(kc == KC - 1),
                    )
                nc.vector.tensor_copy(dest[:, 512 * n:512 * (n + 1)], ps[:])

            def vtrans(t_i):
                vtps = pool.tile([P, P], BF16, name=f"{pfx}vtp{m}_{t_i}", tag="ps")
                nc.tensor.transpose(vtps[:], vT[:, P * t_i:P * (t_i + 1)], ident[:])
                nc.vector.tensor_copy(
                    vav[:, t_i, :, 0:D],
                    vtps[:].rearrange("p (g c) -> p g c", g=2),
                )

            if interleave:
                for n in range(4):
                    for nm in ("k", "q", "v"):
                        block(nm, n)
                    for t_i in range(4 * n, 4 * n + 4):
                        vtrans(t_i)
            else:
                for nm in ("k", "q", "v"):
                    for n in range(4):
                        block(nm, n)
                    if nm == "v":
                        for t_i in range(NT):
                            vtrans(t_i)

        def attn_qb(m, qb, spsp, cpsp):
            """Both heads of pair m for q-block qb: scores, exp, ctx, norm."""
            q0 = QB * qb
            vav = va[m][:].rearrange("p (t g c) -> p t g c", t=NT, g=2)
            cps = [cpsp.tile([D + 1, QB], F32, name=f"{pfx}cps{m}_{qb}_{g}", tag="cps")
                   for g in range(2)]
            for j in range(NT):
                sps = spsp.tile([P, 2 * QB], F32, name=f"{pfx}sps{m}{qb}{j}", tag="sps")
                for g in range(2):
                    nc.tensor.matmul(
                        sps[:, QB * g:QB * (g + 1)],
                        kT[m][D * g:D * (g + 1), P * j:P * (j + 1)],
                        qT[m][D * g:D * (g + 1), q0:q0 + QB],
                        start=True, stop=True,
                    )
                pt = work.tile([P, 2 * QB], BF16, name=f"{pfx}pt{m}{qb}{j}", tag="pt")
                nc.scalar.activation(pt[:], sps[:], AF.Exp, scale=float(D) ** -0.5)
                for g in range(2):
                    nc.tensor.matmul(
                        cps[g][:],
                        vav[:, j, g, 0:D + 1],
                        pt[:, QB * g:QB * (g + 1)],
                        start=(j == 0), stop=(j == NT - 1),
                    )
            # evict ctx + sums out of PSUM fast (frees cps for the next
            # q-block), then normalize from SBUF off the critical path
            for g in range(2):
                s_sb = normp.tile([1, QB], F32, name=f"{pfx}ssb{m}{qb}{g}", tag="ssb")
                nc.vector.tensor_copy(s_sb[:], cps[g][D:D + 1, :])
                cu = normp.tile([D, QB], F32, name=f"{pfx}cu{m}{qb}{g}", tag="cu")
                nc.vector.tensor_copy(cu[:], cps[g][0:D, :])
                rr = normp.tile([1, QB], F32, name=f"{pfx}rr{m}{qb}{g}", tag="rr")
                nc.vector.reciprocal_approx_fast(rr[:], s_sb[:])
                rbc = normp.tile([D, QB], F32, name=f"{pfx}rbc{m}{qb}{g}", tag="rbc")
                nc.gpsimd.partition_broadcast(rbc[:], rr[:])
                nc.vector.tensor_mul(
                    ctxT[m][D * g:D * (g + 1), q0:q0 + QB],
                    cu[:],
                    rbc[:],
                )

        def outproj(t_lo, t_hi, psop):
            for t_i in range(t_lo, t_hi):
                psA = psop.tile([P, 512], F32, name=f"{pfx}psA{t_i}", tag="pso")
                psB = psop.tile([P, C - 512], F32, name=f"{pfx}psB{t_i}", tag="pso")
                for m in range(MC):
                    nc.tensor.matmul(psA[:], ctxT[m][:, P * t_i:P * (t_i + 1)],
                                     wo_b[m][:, 0:512], start=(m == 0), stop=(m == MC - 1))
                for m in range(MC):
                    nc.tensor.matmul(psB[:], ctxT[m][:, P * t_i:P * (t_i + 1)],
                                     wo_b[m][:, 512:C], start=(m == 0), stop=(m == MC - 1))
                ob = outp.tile([P, C], F32, name=f"{pfx}ob{t_i}", tag="ob")
                nc.vector.tensor_copy(ob[:, 0:512], psA[:])
                nc.vector.tensor_copy(ob[:, 512:C], psB[:])
                nc.sync.dma_start(out[P * t_i:P * (t_i + 1), :], ob[:])

        # proj m=0 with a wide psum pool (pre-attention), then attention with
        # proj m=1,2 overlapped through a narrow pool.
        with tc.tile_pool(name=pfx + "psprojA", bufs=2, space="PSUM") as psprojA, \
             tc.tile_pool(name=pfx + "vtpA", bufs=1) as vtpA:
            xphase(psprojA)
            proj_m(0, psprojA, vtpA, interleave=True)
        with tc.tile_pool(name=pfx + "sps", bufs=2, space="PSUM") as spsp, \
             tc.tile_pool(name=pfx + "cps", bufs=2, space="PSUM") as cpsp, \
             tc.tile_pool(name=pfx + "vtpB", bufs=1) as vtpB:
            with tc.tile_pool(name=pfx + "psprojB", bufs=2, space="PSUM") as psprojB:
                attn_qb(0, 0, spsp, cpsp)
                proj_m(1, psprojB, vtpB)
                attn_qb(0, 1, spsp, cpsp)
                attn_qb(1, 0, spsp, cpsp)
                proj_m(2, psprojB, vtpB)
                attn_qb(1, 1, spsp, cpsp)
                attn_qb(2, 0, spsp, cpsp)
                attn_qb(2, 1, spsp, cpsp)
            with tc.tile_pool(name=pfx + "psoA", bufs=2, space="PSUM") as psoA:
                attn_qb(0, 2, spsp, cpsp)
                outproj(0, 4, psoA)
                attn_qb(1, 2, spsp, cpsp)
                outproj(4, 8, psoA)
                attn_qb(2, 2, spsp, cpsp)
                attn_qb(0, 3, spsp, cpsp)
                outproj(8, 12, psoA)
                attn_qb(1, 3, spsp, cpsp)
                attn_qb(2, 3, spsp, cpsp)
        with tc.tile_pool(name=pfx + "psoB", bufs=2, space="PSUM") as psoB:
            outproj(12, NT, psoB)

    with tile.TileContext(nc) as tc, ExitStack() as ctx:
        consts = ctx.enter_context(tc.tile_pool(name="consts", bufs=1))
        ident = consts.tile([P, P], BF16)
        make_identity(nc, ident)
        big = ctx.enter_context(tc.tile_pool(name="big", bufs=16))
        vap = ctx.enter_context(tc.tile_pool(name="vap", bufs=1))
        work = ctx.enter_context(tc.tile_pool(name="work", bufs=4))
        outp = ctx.enter_context(tc.tile_pool(name="outp", bufs=2))
        norm = ctx.enter_context(tc.tile_pool(name="norm", bufs=1))
        normp = ctx.enter_context(tc.tile_pool(name="normp", bufs=2))
        pools = ((ident,), big, vap, work, outp, norm, normp)
        for rep in range(repeat):
            emit(f"r{rep}_", tc, pools)

    nc.compile()
    return nc


def kernel(X, Wq, Wk, Wv, Wo, bo):
    from concourse import bass_utils

    if "nc" not in _cache:
        _cache["nc"] = _build(int(os.environ.get("KERNEL_REPEAT", "1")))
    nc = _cache["nc"]

    X = np.asarray(X, dtype=np.float32)
    in_maps = []
    for c in range(8):
        b, g = divmod(c, 2)
        sl = slice(HD * g, HD * (g + 1))
        in_maps.append({
            "x": np.ascontiguousarray(X[b]),
            "wq": np.ascontiguousarray(np.asarray(Wq, np.float32)[:, sl]),
            "wk": np.ascontiguousarray(np.asarray(Wk, np.float32)[:, sl]),
            "wv": np.ascontiguousarray(np.asarray(Wv, np.float32)[:, sl]),
            "wo": np.ascontiguousarray(np.asarray(Wo, np.float32)[sl, :]),
        })
    trace = bool(os.environ.get("BASS_KERNEL_TRACE"))
    kw = {}
    if trace:
        kw["trace"] = True
        td = os.environ.get("BASS_TRACE_DIR")
        if td:
            os.makedirs(td, exist_ok=True)
            kw["tmpdir"] = td
    res = bass_utils.run_bass_kernel_spmd(nc, in_maps, core_ids=list(range(8)), **kw)
    _cache["last"] = res
    outf = np.empty((4, T, C), np.float32)
    bo = np.asarray(bo, np.float32)
    for b in range(4):
        outf[b] = res.results[2 * b]["out"] + res.results[2 * b + 1]["out"] + bo
    return outf


# revision 29
# speedup vs baseline: 1.0715x; 1.0715x over previous
"""Multi-head self-attention (B=4, T=2048, C=768, H=12) on 8 trn2 NeuronCores.

Sharding: core c -> batch b=c//2, head-group g=c%2 (6 heads each).
Each core computes its 6 heads' attention and a partial output projection
(contraction over its 384 ctx dims). Host sums the 2 partials per batch
and adds the bias.

v2 dataflow (bf16 compute, f32 PSUM accumulate):
  X -> bf16 via gpsimd cast-DMA -> X^T via DMA-XBAR transpose (no PE)
  qT/kT [128, T] bf16 per pair m (2 heads: partitions 0-63 / 64-127)
  v -> va[m][128, t, g, 65] via DMA transpose; col 0 = ones (softmax
  denominators fall out of the ctx matmul row 0)
  attention per (pair m, q-block qb of 512, key chunk j):
    sps[128, 1024] = [h0 scores | h1 scores]   (two K=64 row-tiled MMs)
    exp on ACT -> pt bf16; ctx MMs accumulate cps_h [65, 512]
  normalize: recip(sums) -> DMA partition-broadcast -> DVE mul -> ctxT bf16
  outproj: psA/psB f32 accumulate over m -> DVE copy -> DMA out (partial)

KERNEL_REPEAT=N builds the body N times (for overhead-cancelling timing).
"""
import sys
import os

sys.path.insert(0, "/opt/trn_rl_repo")

import numpy as np

P = 128
T = 2048
C = 768
HD = 384          # per-core head columns (6 heads x 64)
D = 64
NT = T // P       # 16 T chunks of 128
KC = C // P       # 6 contraction chunks for C
MC = HD // P      # 3 pairs of heads (128 head-dims each)
QB = 512          # q-block size for attention
NQB = T // QB     # 4 q-blocks
VC = 80           # per-head va col block (64 v dims + ones + pad, 32B-aligned)
VW = 2 * VC

_cache = {}


def _build(repeat=1):
    import concourse.bacc as bacc
    import concourse.mybir as mybir
    import concourse.tile as tile
    from concourse.masks import make_identity
    from contextlib import ExitStack

    F32 = mybir.dt.float32
    BF16 = mybir.dt.bfloat16
    AF = mybir.ActivationFunctionType
    ALU = mybir.AluOpType

    nc = bacc.Bacc("TRN2", target_bir_lowering=False, debug=False)
    x = nc.dram_tensor("x", [T, C], F32, kind="ExternalInput").ap()
    wq = nc.dram_tensor("wq", [C, HD], F32, kind="ExternalInput").ap()
    wk = nc.dram_tensor("wk", [C, HD], F32, kind="ExternalInput").ap()
    wv = nc.dram_tensor("wv", [C, HD], F32, kind="ExternalInput").ap()
    wo = nc.dram_tensor("wo", [HD, C], F32, kind="ExternalInput").ap()
    out = nc.dram_tensor("out", [T, C], F32, kind="ExternalOutput").ap()

    def emit(pfx, tc, pools):
        (ident,), big, vap, work, outp, norm, normp = pools

        xt = [big.tile([P, T], BF16, name=f"{pfx}xt{kc}", tag="bigT") for kc in range(KC)]
        qT = [big.tile([P, T], BF16, name=f"{pfx}qT{m}", tag="bigT") for m in range(MC)]
        kT = [big.tile([P, T], BF16, name=f"{pfx}kT{m}", tag="bigT") for m in range(MC)]
        va = [vap.tile([P, NT * VW], BF16, name=f"{pfx}va{m}", tag=f"va{m}") for m in range(MC)]
        ctxT = [big.tile([P, T], BF16, name=f"{pfx}ctxT{m}", tag="bigT") for m in range(MC)]
        w_b = {}
        wo_b = []

        # ---- X: HWDGE f32 load -> DVE cast bf16 -> PE transpose -> xt
        def xgroup(tq, pool, xrp, xsp):
            xrs = []
            for i in range(4):
                t_i = 4 * tq + i
                xs = xsp.tile([P, C], F32, name=f"{pfx}xs{t_i}", tag="xs")
                nc.sync.dma_start(xs[:], x[P * t_i:P * (t_i + 1), :])
                xr = xrp.tile([P, C], BF16, name=f"{pfx}xr{t_i}", tag="xr")
                nc.vector.tensor_copy(xr[:], xs[:])
                xrs.append(xr)
            for kc in range(KC):
                tp = pool.tile([P, 512], BF16, name=f"{pfx}tp_{tq}_{kc}", tag="ps")
                for i in range(4):
                    nc.tensor.transpose(tp[:, P * i:P * (i + 1)], xrs[i][:, P * kc:P * (kc + 1)], ident[:])
                nc.vector.tensor_copy(xt[kc][:, 512 * tq:512 * (tq + 1)], tp[:])

        # ---- weights: HWDGE f32 load -> ACT cast (q/k/v) or DVE cast (wo)
        def wload(nm, src, wsp):
            for kc in range(KC):
                st = wsp.tile([P, HD], F32, name=f"{pfx}wst_{nm}{kc}", tag="wst")
                nc.sync.dma_start(st[:], src[P * kc:P * (kc + 1), :])
                t_b = norm.tile([P, HD], BF16, name=f"{pfx}w_{nm}{kc}", tag=f"w_{nm}{kc}")
                nc.scalar.copy(t_b[:], st[:])
                w_b[nm, kc] = t_b

        def wload_wo(wsp):
            for m in range(MC):
                st = wsp.tile([P, C], F32, name=f"{pfx}wst_o{m}", tag="wsto")
                nc.sync.dma_start(st[:], wo[P * m:P * (m + 1), :])
                t_b = norm.tile([P, C], BF16, name=f"{pfx}wo_{m}", tag=f"wo_{m}")
                nc.vector.tensor_copy(t_b[:], st[:])
                wo_b.append(t_b)

        def blocks(m, pool, vtp):
            """Return 4 per-n emitters for pair m's projections (k,q,v + va)."""
            vT = vtp.tile([P, T], BF16, name=f"{pfx}vT{m}", tag="vT")
            vav = va[m][:].rearrange("p (t g c) -> p t g c", t=NT, g=2)
            nc.vector.memset(vav[:, :, :, D:D + 1], 1.0)

            def mkblock(n):
                def go():
                    for nm in ("k", "q", "v"):
                        dest = {"q": qT[m], "k": kT[m], "v": vT}[nm]
                        ps = pool.tile([P, 512], F32, name=f"{pfx}ps_{nm}{m}{n}", tag="ps")
                        for kc in range(KC):
                            nc.tensor.matmul(
                                ps[:],
                                w_b[nm, kc][:, P * m:P * (m + 1)],
                                xt[kc][:, 512 * n:512 * (n + 1)],
                                start=(kc == 0), stop=(kc == KC - 1),
                            )
                        nc.vector.tensor_copy(dest[:, 512 * n:512 * (n + 1)], ps[:])
                    for t_i in range(4 * n, 4 * n + 4):
                        vtps = pool.tile([P, P], BF16, name=f"{pfx}vtp{m}_{t_i}", tag="ps")
                        nc.tensor.transpose(vtps[:], vT[:, P * t_i:P * (t_i + 1)], ident[:])
                        nc.vector.tensor_copy(
                            vav[:, t_i, :, 0:D],
                            vtps[:].rearrange("p (g c) -> p g c", g=2),
                        )
                return go
            return [mkblock(n) for n in range(4)]

        def attn_qb(m, qb, spsp, cpsp, projblocks=None):
            """Both heads of pair m for q-block qb; projblocks[n] (emitted
            before j-range 4n) lets later projections ride the exp stream."""
            q0 = QB * qb
            vav = va[m][:].rearrange("p (t g c) -> p t g c", t=NT, g=2)
            cps = [cpsp.tile([D + 1, QB], F32, name=f"{pfx}cps{m}_{qb}_{g}", tag="cps")
                   for g in range(2)]
            for j in range(NT):
                if projblocks and j % 4 == 0 and projblocks[j // 4] is not None:
                    projblocks[j // 4]()
                sps = spsp.tile([P, 2 * QB], F32, name=f"{pfx}sps{m}{qb}{j}", tag="sps")
                for g in range(2):
                    nc.tensor.matmul(
                        sps[:, QB * g:QB * (g + 1)],
                        kT[m][D * g:D * (g + 1), P * j:P * (j + 1)],
                        qT[m][D * g:D * (g + 1), q0:q0 + QB],
                        start=True, stop=True,
                    )
                pt = work.tile([P, 2 * QB], BF16, name=f"{pfx}pt{m}{qb}{j}", tag="pt")
                nc.scalar.activation(pt[:], sps[:], AF.Exp, scale=float(D) ** -0.5)
                for g in range(2):
                    nc.tensor.matmul(
                        cps[g][:],
                        vav[:, j, g, 0:D + 1],
                        pt[:, QB * g:QB * (g + 1)],
                        start=(j == 0), stop=(j == NT - 1),
                    )
            # evict ctx + sums out of PSUM fast (frees cps for the next
            # q-block), then normalize from SBUF off the critical path
            for g in range(2):
                s_sb = normp.tile([1, QB], F32, name=f"{pfx}ssb{m}{qb}{g}", tag="ssb")
                nc.vector.tensor_copy(s_sb[:], cps[g][D:D + 1, :])
                cu = normp.tile([D, QB], F32, name=f"{pfx}cu{m}{qb}{g}", tag="cu")
                nc.vector.tensor_copy(cu[:], cps[g][0:D, :])
                rr = normp.tile([1, QB], F32, name=f"{pfx}rr{m}{qb}{g}", tag="rr")
                nc.vector.reciprocal_approx_fast(rr[:], s_sb[:])
                rbc = normp.tile([D, QB], F32, name=f"{pfx}rbc{m}{qb}{g}", tag="rbc")
                nc.gpsimd.partition_broadcast(rbc[:], rr[:])
                nc.vector.tensor_mul(
                    ctxT[m][D * g:D * (g + 1), q0:q0 + QB],
                    cu[:],
                    rbc[:],
                )

        def outproj(t_lo, t_hi, psop):
            for t_i in range(t_lo, t_hi):
                psA = psop.tile([P, 512], F32, name=f"{pfx}psA{t_i}", tag="pso")
                psB = psop.tile([P, C - 512], F32, name=f"{pfx}psB{t_i}", tag="pso")
                for m in range(MC):
                    nc.tensor.matmul(psA[:], ctxT[m][:, P * t_i:P * (t_i + 1)],
                                     wo_b[m][:, 0:512], start=(m == 0), stop=(m == MC - 1))
                for m in range(MC):
                    nc.tensor.matmul(psB[:], ctxT[m][:, P * t_i:P * (t_i + 1)],
                                     wo_b[m][:, 512:C], start=(m == 0), stop=(m == MC - 1))
                ob = outp.tile([P, C], F32, name=f"{pfx}ob{t_i}", tag="ob")
                nc.vector.tensor_copy(ob[:, 0:512], psA[:])
                nc.vector.tensor_copy(ob[:, 512:C], psB[:])
                nc.sync.dma_start(out[P * t_i:P * (t_i + 1), :], ob[:])

        with tc.tile_pool(name=pfx + "sps", bufs=2, space="PSUM") as spsp, \
             tc.tile_pool(name=pfx + "cps", bufs=2, space="PSUM") as cpsp:
            with tc.tile_pool(name=pfx + "psprojA", bufs=2, space="PSUM") as psprojA, \
                 tc.tile_pool(name=pfx + "vtpA", bufs=1) as vtpA, \
                 tc.tile_pool(name=pfx + "xrp", bufs=5) as xrp, \
                 tc.tile_pool(name=pfx + "xsp", bufs=3) as xsp, \
                 tc.tile_pool(name=pfx + "wsp", bufs=4) as wsp:
                xgroup(0, psprojA, xrp, xsp)
                wload("k", wk, wsp)
                xgroup(1, psprojA, xrp, xsp)
                wload("q", wq, wsp)
                xgroup(2, psprojA, xrp, xsp)
                wload("v", wv, wsp)
                xgroup(3, psprojA, xrp, xsp)
                A = blocks(0, psprojA, vtpA)
                attn_qb(0, 0, spsp, cpsp, projblocks=A)
            with tc.tile_pool(name=pfx + "psprojB", bufs=2, space="PSUM") as psprojB, \
                 tc.tile_pool(name=pfx + "vtpB", bufs=1) as vtpB, \
                 tc.tile_pool(name=pfx + "wspo", bufs=2) as wspo:
                wload_wo(wspo)
                B = blocks(1, psprojB, vtpB)
                attn_qb(0, 1, spsp, cpsp, projblocks=[B[0], B[1], None, None])
                attn_qb(1, 0, spsp, cpsp, projblocks=[B[2], B[3], None, None])
                Cb = blocks(2, psprojB, vtpB)
                attn_qb(1, 1, spsp, cpsp, projblocks=[Cb[0], Cb[1], None, None])
                attn_qb(2, 0, spsp, cpsp, projblocks=[Cb[2], Cb[3], None, None])
                attn_qb(2, 1, spsp, cpsp)
            with tc.tile_pool(name=pfx + "psoA", bufs=2, space="PSUM") as psoA:
                attn_qb(0, 2, spsp, cpsp)
                outproj(0, 4, psoA)
                attn_qb(1, 2, spsp, cpsp)
                outproj(4, 8, psoA)
                attn_qb(2, 2, spsp, cpsp)
                attn_qb(0, 3, spsp, cpsp)
                outproj(8, 12, psoA)
                attn_qb(1, 3, spsp, cpsp)
                attn_qb(2, 3, spsp, cpsp)
        with tc.tile_pool(name=pfx + "psoB", bufs=2, space="PSUM") as psoB:
            outproj(12, NT, psoB)

    with tile.TileContext(nc) as tc, ExitStack() as ctx:
        consts = ctx.enter_context(tc.tile_pool(name="consts", bufs=1))
        ident = consts.tile([P, P], BF16)
        make_identity(nc, ident)
        big = ctx.enter_context(tc.tile_pool(name="big", bufs=16))
        vap = ctx.enter_context(tc.tile_pool(name="vap", bufs=1))
        work = ctx.enter_context(tc.tile_pool(name="work", bufs=4))
        outp = ctx.enter_context(tc.tile_pool(name="outp", bufs=2))
        norm = ctx.enter_context(tc.tile_pool(name="norm", bufs=1))
        normp = ctx.enter_context(tc.tile_pool(name="normp", bufs=2))
        pools = ((ident,), big, vap, work, outp, norm, normp)
        for rep in range(repeat):
            emit(f"r{rep}_", tc, pools)

    nc.compile()
    return nc


def kernel(X, Wq, Wk, Wv, Wo, bo):
    from concourse import bass_utils

    if "nc" not in _cache:
        _cache["nc"] = _build(int(os.environ.get("KERNEL_REPEAT", "1")))
    nc = _cache["nc"]

    X = np.asarray(X, dtype=np.float32)
    in_maps = []
    for c in range(8):
        b, g = divmod(c, 2)
        sl = slice(HD * g, HD * (g + 1))
        in_maps.append({
            "x": np.ascontiguousarray(X[b]),
            "wq": np.ascontiguousarray(np.asarray(Wq, np.float32)[:, sl]),
            "wk": np.ascontiguousarray(np.asarray(Wk, np.float32)[:, sl]),
            "wv": np.ascontiguousarray(np.asarray(Wv, np.float32)[:, sl]),
            "wo": np.ascontiguousarray(np.asarray(Wo, np.float32)[sl, :]),
        })
    trace = bool(os.environ.get("BASS_KERNEL_TRACE"))
    kw = {}
    if trace:
        kw["trace"] = True
        td = os.environ.get("BASS_TRACE_DIR")
        if td:
            os.makedirs(td, exist_ok=True)
            kw["tmpdir"] = td
    res = bass_utils.run_bass_kernel_spmd(nc, in_maps, core_ids=list(range(8)), **kw)
    _cache["last"] = res
    outf = np.empty((4, T, C), np.float32)
    bo = np.asarray(bo, np.float32)
    for b in range(4):
        outf[b] = res.results[2 * b]["out"] + res.results[2 * b + 1]["out"] + bo
    return outf
